# revision 13
# baseline (speedup 1.0000x reference)
"""Gaussian-mixture log-likelihood kernel for Trainium2 (8 NeuronCores), v4.

Computes ll[i] = logsumexp_j( wlog[j] - (x_i-mu_j)^T G_j (x_i-mu_j) ),
G_j = A_j A_j^T / 2, wlog = log_softmax(w) + 0.5*log(det(G_j)),
for sample (N,2), mu (M,2), A (M,2,2), w (M,1), N=131072, M=2048.

v4 design ("retrieval" pruning; ~3x over the v3 full-evaluation kernel):

  * The v3 kernel is ScalarE-bound: N*M = 33.5M exps/core at 1 elem/cycle/
    lane/1.2GHz is a ~220us floor.  v4 reduces exp count: samples are
    Morton-sorted on host into spatially tight tiles of 128; for each tile
    only the components that can contribute to any of its samples'
    logsumexp (within a rigorously bounded drop-mass threshold, 64-rounded,
    capped at 1536) are evaluated.  Mean K ~ 450 of 2048.
  * Per-sample exact shift: host computes vlb_i = wlog_j* - q(x_i, mu_j*)
    for the euclidean-nearest component j* (exact max for this model family
    since all G_j are equal+isotropic; a valid lower bound in general).
    The exp bias is DMA'd per sample, so the device needs NO row-max
    reduce: the main loop runs matmuls + ONE Exp activation per tile.
    logsumexp is shift-exact, so any in-range bias gives the right answer.
  * Tile-centered coordinates (y = x - c_tile) plus the v3 stacked
    residual trick recover full fp32 precision from FP32R matmuls
    (PE rounding = RNE to 11 explicit mantissa bits, measured): with
    phi = [y0^2, y0*y1, y1^2, y0, y1, 1] and theta the matching rank-6
    coefficients (m = mu - c_tile),
        v = [phi_r, phi_res, phi_r](18) . [th_r; th_r; th_res]
    where *_r = rne11 rounding and *_res the residual; th_r/th_res are
    pre-rounded on host (11-bit values pass the PE untouched), phi_r and
    its residual are materialized on device by f32r-rounding DVE writes.
    Matmul cost is unchanged (PE time scales with output width, not
    contraction rows), and theta rows live in otherwise-unused partitions
    of the same SBUF cols.
  * Per-tile theta AND the transposed phi stacks are gathered/packed on
    host (phi pre-rounding is exact: the PE f32r rounding was measured as
    RNE-11 and 11-bit values pass through unchanged), so the device does
    NO transposes, identity build or phi prep at all -- the whole kernel
    is matmuls + one Exp per tile.  Slots are sorted by K ascending and
    dealt round-robin to the 8 cores so all cores share one SPMD program
    with balanced work; ascending order lets the early tiles start on
    ~KB-scale DMAs while the fat tail chunks stream in behind.
  * Matmul output chunks are 512-aligned in PSUM (bank-aligned, HW
    requirement); K<=1536 fits 3 banks x2 bufs + transpose bank x2.
  * Host work is O(N log M + NT*M) numpy indexing (sort, bbox distances,
    nearest-neighbor query): the N*M score evaluation, exp and sum all
    stay on device.

Steady state: ACT busy ~ sum_t (K_t * 0.83ns + ~400ns) ~ 95us/core.
"""

import sys

import numpy as np

sys.path.insert(0, "/opt/trn_rl_repo")

import concourse.bass as bass
import concourse.bacc as bacc
import concourse.mybir as mybir
from concourse.tile import TileContext
from concourse.bass_utils import run_bass_kernel_spmd
from concourse.masks import make_identity

N_FULL, M, D = 131072, 2048, 2
NCORES = 8
NSH = N_FULL // NCORES          # samples per core
P = 128                          # partitions
T = NSH // P                     # 128 tile-slots per core
NT = N_FULL // P                 # 1024 global tiles
KGRAN = 64                       # K rounding granularity
NK = 19                          # stacked contraction rows + bias row
KCAP = 1024                      # 2 PSUM banks x 4 bufs
TOL = 0.01                       # drop-mass tolerance (vs 2e-2 budget)
NG = 32                          # phiT groups (4 slots each)
BIAS_MARGIN = 3.0                # exp(v - vlb - margin)
NQ = T // 16                     # 8 theta chunk groups (16 slots each)

f32 = mybir.dt.float32
f32r = mybir.dt.float32r
AF = mybir.ActivationFunctionType
ALU = mybir.AluOpType
AX = mybir.AxisListType


# --------------------------------------------------------------------------
# device kernel
# --------------------------------------------------------------------------

def build_kernel(cfg):
    """cfg: dict with K_slot (tuple of 128 ints), w_qs (8x4 chunk widths)."""
    K_slot = cfg["K_slot"]
    n2 = cfg["n2"]
    w_qs = cfg["w_qs"]              # [NQ][4] stream widths per chunk
    wq = [max(ws) for ws in w_qs]   # chunk tile width

    nc = bacc.Bacc(
        "TRN2",
        target_bir_lowering=False,
        debug=False,
        num_devices=NCORES,
    )

    bias_e = nc.declare_dram_parameter("biasp", [P, T], f32, isOutput=False)
    WTOT = sum(K_slot)
    th_e = nc.declare_dram_parameter("thetap", [NK, WTOT], f32r, isOutput=False)
    phit_e = nc.declare_dram_parameter("phitp", [NG * P, P], f32r, isOutput=False)
    out_e = nc.declare_dram_parameter("out", [NSH, 1], f32, isOutput=True)

    with TileContext(nc) as tc:
        with (
            tc.tile_pool(name="singles", bufs=1) as sing,
            tc.tile_pool(name="psum", bufs=4, space="PSUM") as psum,
        ):
            V = nc.vector

            bias_sb = sing.tile([P, T], f32, tag="bias", name="bias")

            # preload the Exp ACT table (1.3us) while DMAs stream in, so
            # the first real EXP doesn't pay for it on the critical path
            warm = sing.tile([P, 4], f32, tag="warm", name="warm")
            V.memset(warm[:], 0.0)
            nc.scalar.activation(warm[:, 0:1], warm[:, 0:1], AF.Exp)

            # All remaining inputs are per-slot/per-group blocks, issued in
            # first-use order alternating between the SP and Pool DMA
            # sequencers (~0.6-1us issue each; a single queue would delay
            # the tail).  phiT group g is used at t=4g; theta chunk (q,s)
            # at t=16q+s.
            phiTg = [
                sing.tile([P, P], f32r, tag=f"phiT{g}", name=f"phiT{g}")
                for g in range(NG)
            ]
            th_sb = [sing.tile([P, wq[q]], f32r, tag=f"th{q}", name=f"th{q}") for q in range(NQ)]

            # theta dram is packed as contiguous (q, s) stream blocks:
            # block (q, s) holds slots 16q+s, 16q+4+s, 16q+8+s, 16q+12+s
            th_off = {}
            goff = 0
            for q in range(NQ):
                for s in range(4):
                    th_off[(q, s)] = goff
                    goff += sum(K_slot[16 * q + 4 * j + s] for j in range(4))

            dmas = []
            for q in range(NQ):
                for i in range(4):
                    g = 4 * q + i
                    dmas.append(("phit", g))
                    dmas.append(("theta", (q, i)))
                if q == 0:
                    dmas.append(("bias", None))
            eng_i = 0
            for kind, arg in dmas:
                # the q=0 blocks gate the first tiles: keep them off the
                # slow SWDGE queue (~0.8us/descriptor-gen on Pool)
                early = (kind == "phit" and arg < 8) or (
                    kind == "theta" and arg[0] == 0
                )
                eng = nc.sync if (early or eng_i % 2 == 0) else nc.gpsimd
                eng_i += 1
                if kind == "bias":
                    eng.dma_start(out=bias_sb[:], in_=bias_e[:])
                elif kind == "phit":
                    g = arg
                    eng.dma_start(
                        out=phiTg[g][:], in_=phit_e[g * P : (g + 1) * P, :]
                    )
                else:
                    q, s = arg
                    w = w_qs[q][s]
                    eng.dma_start(
                        out=th_sb[q][32 * s : 32 * s + NK, 0:w],
                        in_=th_e[:, th_off[(q, s)] : th_off[(q, s)] + w],
                    )

            # main loop
            sa_all = sing.tile([P, T], f32, tag="sa_all")

            def slot_mm(S, t, seg):
                # matmuls for slot t into S at col offset 512*seg; bias is
                # row 18 of the stack (per-sample phi values x ones theta)
                g, s = t // 4, t % 4
                q, j = t // 16, (t % 16) // 4
                off = sum(K_slot[16 * q + 4 * jj + s] for jj in range(j))
                K = K_slot[t]
                lhsT = phiTg[g][32 * s : 32 * s + NK, :]
                chunks = [(c0, min(512, K - c0)) for c0 in range(0, K, 512)]
                for c0, w in sorted(chunks, key=lambda cw: cw[1]):
                    nc.tensor.matmul(
                        S[:, 512 * seg + c0 : 512 * seg + c0 + w],
                        lhsT,
                        th_sb[q][32 * s : 32 * s + NK, off + c0 : off + c0 + w],
                        start=True,
                        stop=True,
                        tile_position=(32 * s, 0),
                    )

            # merged prefix: pairs of equal-K slots share one PSUM tile;
            # one Exp covers both segments via a [P, 2, K] view and the
            # (idle) DVE does the segmented row sums, replacing two
            # accumulator reads and one Exp issue per pair
            t = 0
            while t < T:
                if t < n2:
                    K = K_slot[t]
                    S = psum.tile([P, KCAP], f32, tag="S", name=f"S{t}")
                    slot_mm(S, t, 0)
                    slot_mm(S, t + 1, 1)
                    view = S[:].rearrange("p (g k) -> p g k", g=2)[:, :, 0:K]
                    nc.scalar.activation(view, view, AF.Exp)
                    V.tensor_reduce(
                        sa_all[:, t : t + 2], view, axis=AX.X, op=ALU.add
                    )
                    t += 2
                else:
                    K = K_slot[t]
                    S = psum.tile([P, KCAP], f32, tag="S", name=f"S{t}")
                    slot_mm(S, t, 0)
                    nc.scalar.activation(
                        S[:, 0:K],
                        S[:, 0:K],
                        AF.Exp,
                        accum_out=sa_all[:, t : t + 1],
                    )
                    t += 1

            # tail: ll = ln(sum) - bias
            ls_all = sing.tile([P, T], f32, tag="ls_all")
            ll_all = sing.tile([P, T], f32, tag="ll_all")
            nc.scalar.activation(ls_all[:], sa_all[:], AF.Ln)
            V.tensor_tensor(ll_all[:], ls_all[:], bias_sb[:], ALU.subtract)
            nc.sync.dma_start(
                out=out_e[:].rearrange("(p t) o -> p (t o)", p=P),
                in_=ll_all[:],
            )

    nc.compile()
    return nc


# --------------------------------------------------------------------------
# host-side preparation
# --------------------------------------------------------------------------

def _rne11(x):
    """Round float32 array to 11 explicit mantissa bits, RNE (PE f32r model)."""
    xi = np.asarray(x, np.float32).view(np.int32)
    drop = 12
    half = (1 << (drop - 1)) - 1
    return ((xi + half + ((xi >> drop) & 1)) >> drop << drop).view(np.float32)


def _hilbert_order(x, bits=16):
    """Sort 2-D points along a Hilbert curve (tighter tiles than Morton)."""
    lo = x.min(0)
    hi = x.max(0)
    n = 1 << bits
    px = np.minimum((x[:, 0] - lo[0]) / (hi[0] - lo[0] + 1e-9) * n, n - 1).astype(np.uint64)
    py = np.minimum((x[:, 1] - lo[1]) / (hi[1] - lo[1] + 1e-9) * n, n - 1).astype(np.uint64)
    rx = np.zeros_like(px)
    ry = np.zeros_like(py)
    d = np.zeros_like(px)
    s = np.uint64(1 << (bits - 1))
    while s > 0:
        rx = ((px & s) > 0).astype(np.uint64)
        ry = ((py & s) > 0).astype(np.uint64)
        d += s * s * ((np.uint64(3) * rx) ^ ry)
        # rotate
        swap = ry == 0
        flip = swap & (rx == 1)
        px_f = np.where(flip, s - 1 - px, px)
        py_f = np.where(flip, s - 1 - py, py)
        px, py = np.where(swap, py_f, px_f), np.where(swap, px_f, py_f)
        s >>= np.uint64(1)
    return np.argsort(d, kind="stable")


def _nearest(mu, x):
    """Index of euclidean-nearest mu row for each x row."""
    try:
        from scipy.spatial import cKDTree

        return cKDTree(mu).query(x, k=1)[1]
    except Exception:
        jj = np.empty(x.shape[0], np.int64)
        for i in range(0, x.shape[0], 8192):
            sl = slice(i, i + 8192)
            d2 = ((x[sl, None, :] - mu[None, :, :]) ** 2).sum(-1)
            jj[sl] = np.argmin(d2, axis=1)
        return jj


def _prepare(sample, mu, A, w):
    """Returns (cfg, in_maps_extra, unpack) for the given full inputs."""
    s64 = sample.astype(np.float64)
    mu64 = mu.astype(np.float64)
    A64 = A.astype(np.float64)
    w64 = w.astype(np.float64)

    A00, A01 = A64[:, 0, 0], A64[:, 0, 1]
    A10, A11 = A64[:, 1, 0], A64[:, 1, 1]
    s0 = A00 * A00 + A01 * A01
    s1 = A10 * A10 + A11 * A11
    s01 = A00 * A10 + A01 * A11
    qa, qb, qc = s0 / 2, s01, s1 / 2          # q = qa dx0^2 + qb dx0 dx1 + qc dx1^2
    det4 = s0 * s1 - s01 * s01
    wl = w64[:, 0]
    lse = np.log(np.exp(wl - wl.max()).sum()) + wl.max()
    wlog = (wl - lse) + 0.5 * np.log(det4) - np.log(2.0)
    tr = qa + qc
    disc = np.sqrt(np.maximum((qa - qc) ** 2 + qb * qb, 0.0))
    lmin = (tr - disc) / 2                     # min eigenvalue of G_j

    # per-sample exact shift (lower bound on vmax in general)
    jj = _nearest(mu64, s64)
    dx0 = s64[:, 0] - mu64[jj, 0]
    dx1 = s64[:, 1] - mu64[jj, 1]
    vlb = wlog[jj] - (qa[jj] * dx0 * dx0 + qb[jj] * dx0 * dx1 + qc[jj] * dx1 * dx1)

    order = _hilbert_order(s64)
    s_sorted = s64[order]
    vlb_s = vlb[order].reshape(NT, P)
    tiles = s_sorted.reshape(NT, P, D)
    blo = tiles.min(1)
    bhi = tiles.max(1)
    ctr = (blo + bhi) / 2                      # (NT, 2)

    d0 = np.maximum(np.maximum(blo[:, None, 0] - mu64[None, :, 0], mu64[None, :, 0] - bhi[:, None, 0]), 0.0)
    d1 = np.maximum(np.maximum(blo[:, None, 1] - mu64[None, :, 1], mu64[None, :, 1] - bhi[:, None, 1]), 0.0)
    ub = wlog[None, :] - lmin[None, :] * (d0 * d0 + d1 * d1)   # (NT, M)

    tol_i = TOL * np.maximum(1.0, np.abs(vlb_s) - 8.0)
    log_rhs = (np.log(tol_i) + vlb_s).min(1)
    ub_sorted = np.sort(ub, axis=1)
    mx = ub_sorted[:, -1:]
    with np.errstate(divide="ignore"):
        log_csum = np.log(np.cumsum(np.exp(ub_sorted - mx), axis=1)) + mx
    ndrop = (log_csum <= log_rhs[:, None]).sum(1)
    keep = M - ndrop
    K = np.clip(np.ceil(keep / KGRAN).astype(int) * KGRAN, KGRAN, KCAP)

    # deal tiles to slots: sorted by K desc, slot t gets ranks 8t..8t+8
    t_order = np.argsort(K, kind="stable")
    # slot schedule: keep the pipeline-fill slots small, then alternate
    # small/big so ACT-heavy small tiles donate pipeline slack to the
    # PE-heavy big tiles (PE first-chunk runs at cold p-state)
    rank_of_slot = np.arange(T)
    K_rank = np.array([K[t_order[8 * r + 7]] for r in range(T)], dtype=int)
    K_slot = K_rank[rank_of_slot]
    # merge-pair prefix: adjacent slots with K <= 512 share one PSUM tile
    # (segments at cols 0 and 512) and ONE Exp covering both via a regular
    # [P, 2, K] AP -- requires equal K within a pair
    n2 = int((K_slot <= 512).sum()) & ~1
    for t_ in range(0, n2, 2):
        K_slot[t_] = K_slot[t_ + 1] = max(K_slot[t_], K_slot[t_ + 1])

    # chunk widths
    w_qs = [
        [int(sum(K_slot[16 * q + 4 * j + s] for j in range(4))) for s in range(4)]
        for q in range(NQ)
    ]

    cfg = {"K_slot": tuple(int(k) for k in K_slot), "w_qs": w_qs, "n2": n2}

    # ---------------- per-core packed arrays ----------------
    in_maps = []
    unpack_idx = np.empty((NCORES, P, T), np.int64)
    for c_ in range(NCORES):
        gts = t_order[8 * rank_of_slot + c_]               # global tile per slot
        biasp = np.ascontiguousarray(
            _rne11((-vlb_s[gts].T - BIAS_MARGIN).astype(np.float32))
        )                                                  # (P, T)
        unpack_idx[c_] = order[gts[None, :] * P + np.arange(P)[:, None]]

        WTOT = int(sum(K_slot))
        thetap = np.zeros((NK, WTOT), np.float32)
        thetap[18, :] = 1.0
        phitp = np.zeros((NG * P, P), np.float32)
        goff = 0
        for q_ in range(NQ):
          for s__ in range(4):
           for j_ in range(4):
            t = 16 * q_ + 4 * j_ + s__
            gt = gts[t]
            Kt = int(K_slot[t])
            sel = np.argpartition(-ub[gt], Kt - 1)[:Kt]
            m0 = mu64[sel, 0] - ctr[gt, 0]
            m1 = mu64[sel, 1] - ctr[gt, 1]
            th64 = np.stack([
                -qa[sel], -qb[sel], -qc[sel],
                s0[sel] * m0 + s01[sel] * m1,
                s1[sel] * m1 + s01[sel] * m0,
                wlog[sel] - (qa[sel] * m0 * m0 + qb[sel] * m0 * m1 + qc[sel] * m1 * m1),
            ])                                              # (6, Kt) fp64
            thr = _rne11(th64.astype(np.float32))
            tres = (th64 - thr.astype(np.float64)).astype(np.float32)
            thetap[0:6, goff : goff + Kt] = thr
            thetap[6:12, goff : goff + Kt] = thr
            thetap[12:18, goff : goff + Kt] = tres
            goff += Kt
            # phiT strip for this slot (group g=t//4, rows 32s..32s+18)
            g_, s_ = t // 4, t % 4
            yv = (tiles[gt] - ctr[gt][None, :]).astype(np.float32)   # (P, 2)
            y0, y1 = yv[:, 0], yv[:, 1]
            phi32 = np.stack([y0 * y0, y0 * y1, y1 * y1, y0, y1,
                              np.ones(P, np.float32)])               # (6, P)
            phr = _rne11(phi32)
            pres = _rne11((phi32 - phr).astype(np.float32))
            r0 = g_ * P + 32 * s_
            phitp[r0 : r0 + 6] = phr
            phitp[r0 + 6 : r0 + 12] = pres
            phitp[r0 + 12 : r0 + 18] = phr
            phitp[r0 + 18] = biasp[:, t]
        in_maps.append({"biasp": biasp, "thetap": thetap, "phitp": phitp})

    return cfg, in_maps, unpack_idx


_NC_CACHE = {}


def _get_nc(cfg):
    key = (cfg["K_slot"],)
    if key not in _NC_CACHE:
        _NC_CACHE[key] = build_kernel(cfg)
    return _NC_CACHE[key]


def _run(sample, mu, A, w, trace=False, mm_dtype_name="float32"):
    sample = np.ascontiguousarray(np.asarray(sample, dtype=np.float32))
    mu = np.ascontiguousarray(np.asarray(mu, dtype=np.float32))
    A = np.ascontiguousarray(np.asarray(A, dtype=np.float32))
    w = np.ascontiguousarray(np.asarray(w, dtype=np.float32))
    cfg, in_maps, unpack_idx = _prepare(sample, mu, A, w)
    nc = _get_nc(cfg)
    res = run_bass_kernel_spmd(nc, in_maps, list(range(NCORES)), trace=trace)
    out = np.empty((N_FULL, 1), np.float32)
    for c_ in range(NCORES):
        ll = res.results[c_]["out"].reshape(P, T)
        out[unpack_idx[c_].reshape(-1), 0] = ll.reshape(-1)
    return out, res


def kernel(sample, mu, A, w):
    out, _ = _run(sample, mu, A, w, trace=False)
    return out
